# revision 21
# baseline (speedup 1.0000x reference)
"""AttentionBlock (GroupNorm + 1x1-conv QKV self-attention + residual) on 8 TRN2 cores.

Sharding: data-parallel over batch B=4 x sequence-parallel over the 4096
tokens (2 cores per batch element, each handling 2048 query rows; each core
receives x with ITS query tokens permuted to the front — attention sums and
GroupNorm stats are permutation-invariant over tokens, so one SPMD program
serves both halves).

Key algebraic fold (v2): since softmax columns sum to 1,
  proj = wp @ (wv h + bv 1^T) @ P_norm + bp 1^T
       = (wp wv) @ (h @ P_norm) + (wp bv + bp) 1^T
  h @ P_norm = scale ⊙ (x @ P_norm) + shift 1^T
so with W = wp wv (precomputed on host), W' = W diag(scale):
  proj = W' @ (x @ P)/den + [W shift + wp bv + bp]
The whole V path (v production, V^T materialization + PSUM->SBUF copy)
disappears; the P*V matmul becomes x^T-stationary (x ships in BOTH
channel-major fp8 (x8) and token-major fp8 (x8t) layouts, 1MB each).
W' ships x64 pre-scaled so its fp8 encoding avoids denormals; the 1/64
folds into the output residual add.

exp runs on TWO engines: the ACT engine computes exp(s*scale)/4 -> fp8
directly (bias=-ln4), and a fraction of tiles run on the DVE via the
Schraudolph bit trick: fp8e4m3 bits are linear in log2(v), so
int8(s*A + B) bit-viewed as fp8 IS exp(s*scale)/4 to ~±3% — one
tensor_scalar instruction per tile, straight from the S^T PSUM. The /4
keeps the bit pattern clear of the fp8 NaN region with ~6x range margin
(s*scale measured in ±0.8); numerator and denominator share the /4 so
softmax is unaffected.

Per-core device kernel:
  - GroupNorm stats chase the x8 DMA split across DVE(bn_stats)+ACT
    (accumulate), one indicator-matmul pair combines groups; GN folds
    into the QKV weights and W'.
  - q/k: packed [wq'|wk'|wq'|wk'] fp8 DoubleRow matmul per 512-token
    chunk, partition-rearranged into q_rep/k_rep via SBUF->SBUF DMAs.
  - S^T: 2x row-packed bf16 matmuls (K=32) into a 2-bank PSUM ring; exp
    evacuates to fp8 on ACT or DVE per a static round-robin.
  - denominator: fp8 DoubleRow ones-matmul replicates column sums on all
    128 partitions; one reciprocal_approx_fast gives broadcast 1/den.
  - x@P: fp8 DoubleRow, two 128-key m-blocks per pass, x8t stationary,
    accumulated over all 32 m-blocks into a 2-bank PSUM tile per chunk.
  - epilogue per chunk: evac (x@P)*(1/den) -> fp8, ONE DoubleRow matmul
    per 128-channel strip with W'8, o = pj/64 + (x + xc + ball), DMA out.
  - The (chunk, group) iteration space is one software-pipelined stream;
    the S^T/exp front runs LAG groups ahead of the rs/xP drain across
    chunk boundaries; epilogue strips ride the next chunk's slack.
"""
import math
import sys

sys.path.insert(0, "/opt/trn_rl_repo")

import ml_dtypes
import numpy as np

import concourse.tile as tile
from concourse import bacc, mybir
from concourse.bass_utils import run_bass_kernel_spmd

F32 = mybir.dt.float32
BF16 = mybir.dt.bfloat16
FP8 = mybir.dt.float8e4
I8 = mybir.dt.int8

B, C, H, W = 4, 256, 64, 64
N = H * W          # 4096 tokens
NQ = N // 2        # 2048 query rows per core
D = C // 8         # 32 qk dim
G = 32             # groups
GS = C // G        # 8 channels per group
EPS = 1e-5
P = 128            # partitions
CT = C // P        # 2 channel tiles
CH = 512           # nq chunk
NCH = NQ // CH     # 4 chunks
MB = 128           # m block
NMB = N // MB      # 32 m blocks
NG2 = NMB // 2     # 16 groups of 2 m-blocks
SM_SCALE = float(D) ** -0.5
DR = mybir.MatmulPerfMode.DoubleRow

# Schraudolph-to-fp8e4m3: bits = 8*(log2(v) + 7 - CORR), v = exp(s*SM)/4
SCH_CORR = 0.0430
SCH_A = 8.0 * SM_SCALE * math.log2(math.e)
SCH_B = 8.0 * (5.0 - SCH_CORR)
# which group-steps (k % 16) evacuate on the DVE instead of ACT
DVE_EXP_STEPS = (1, 4, 7, 10, 13, 15)

_CACHE = {}
_last_in_maps = None


def _build():
    if "nc" in _CACHE:
        return _CACHE["nc"]

    nc = bacc.Bacc("TRN2", target_bir_lowering=False, debug=False, num_devices=8)

    # x ships as fp8 in TWO layouts: channel-major x8 [p=channel, chunk, ct,
    # CH] (GN stats, qk production, residual) and token-major x8t
    # [p=token-in-block, mb, c] (stationary operand of x@P);
    # xc carries the bf16 residual correction x - fp8(x) for the query half
    x8_ext = nc.declare_dram_parameter("x8", [P, N * CT], FP8, isOutput=False)
    x8t_ext = nc.declare_dram_parameter("x8t", [P, NMB * C], FP8, isOutput=False)
    xc_ext = nc.declare_dram_parameter("xc", [C, NQ], BF16, isOutput=False)
    wqt_ext = nc.declare_dram_parameter("wqt", [C, D], F32, isOutput=False)
    wkt_ext = nc.declare_dram_parameter("wkt", [C, D], F32, isOutput=False)
    # (64 * wp @ wv)^T
    wmt_ext = nc.declare_dram_parameter("wmt", [C, C], F32, isOutput=False)
    # packed per-channel vectors: [gamma, beta, wp@bv+bp]
    gbvp_ext = nc.declare_dram_parameter("gbvp", [C, 4], F32, isOutput=False)
    bqk_ext = nc.declare_dram_parameter("bqk", [D, 2], F32, isOutput=False)
    ind16_ext = nc.declare_dram_parameter("ind16", [P, G // CT], F32, isOutput=False)
    indb_ext = nc.declare_dram_parameter("indb", [G // CT, P], F32, isOutput=False)
    out_ext = nc.declare_dram_parameter("out", [C, NQ], F32, isOutput=True)

    GT = G // CT  # 16 groups per channel tile

    with tile.TileContext(nc) as tc:
        with tc.tile_pool(name="const", bufs=1) as const, \
             tc.tile_pool(name="small", bufs=1) as small:
            # tiny critical constants FIRST on the fast HWDGE sync queue —
            # their data is a few KB and lands before the x8 stream saturates
            # the SDMA engines; queuing them after bulk cost ~10us before.
            ind16_sb = small.tile([P, GT], F32, tag="ind16")
            nc.sync.dma_start(out=ind16_sb, in_=ind16_ext[:])
            indb_sb = small.tile([GT, P], F32, tag="indb")
            nc.sync.dma_start(out=indb_sb, in_=indb_ext[:])
            gbvp_sb = []
            for t in range(CT):
                tl = small.tile([P, 4], F32, tag=f"gbvp{t}", name=f"gbvp{t}")
                nc.sync.dma_start(out=tl, in_=gbvp_ext[t * P:(t + 1) * P, :])
                gbvp_sb.append(tl)
            bqk_sb = small.tile([D, 2], F32, tag="bqk")
            nc.sync.dma_start(out=bqk_sb, in_=bqk_ext[:])
            gamma_sb = [gbvp_sb[t][:, 0:1] for t in range(CT)]
            beta_sb = [gbvp_sb[t][:, 1:2] for t in range(CT)]
            bfix_sb = [gbvp_sb[t][:, 2:3] for t in range(CT)]
            bq_sb = bqk_sb[:, 0:1]
            bk_sb = bqk_sb[:, 1:2]

            # x8 next, split across BOTH fast HWDGE queues (sync + scalar):
            # 4 pieces with 2KB contiguous lines per partition.  The
            # GroupNorm stats chase the pieces.  Bulk transfers (x8t, xc,
            # wmt) are deferred so x8 gets the full SDMA bandwidth.
            NP8 = 4                      # x8 dma pieces
            TP8 = N // NP8               # tokens per piece (1024)
            x8 = const.tile([P, NP8, CT, TP8], FP8, tag="x8", name="x8")
            PCH = CT * TP8               # flattened columns per piece
            for pc in range(NP8):
                qeng = nc.sync if pc % 2 == 0 else nc.scalar
                qeng.dma_start(
                    out=x8[:, pc, :, :],
                    in_=x8_ext[:, pc * PCH:(pc + 1) * PCH].rearrange(
                        "p (t b) -> p t b", t=CT))

            # ---- persistent weight tiles ----
            wqt_sb, wkt_sb = [], []
            for t in range(CT):
                cs = slice(t * P, (t + 1) * P)
                w1 = const.tile([P, D], F32, tag=f"wqt{t}", name=f"wqt{t}")
                nc.sync.dma_start(out=w1, in_=wqt_ext[cs, :])
                wqt_sb.append(w1)
                w2 = const.tile([P, D], F32, tag=f"wkt{t}", name=f"wkt{t}")
                nc.sync.dma_start(out=w2, in_=wkt_ext[cs, :])
                wkt_sb.append(w2)
            # the big transfers (x8t, wmt, xc) are emitted after the stats
            # loop on the scalar queue, so neither rearrange queue (sync=k,
            # gpsimd=q) ever has a bulk transfer in front of it
            wmt_sb = [const.tile([P, C], F32, tag=f"wmt{t}", name=f"wmt{t}")
                      for t in range(CT)]
            x8t = const.tile([P, NMB, C], FP8, tag="x8t", name="x8t")
            # bf16 copies for the (small) effective-bias matmuls
            wqt_hb = [const.tile([P, D], BF16, tag=f"wqthb{t}", name=f"wqthb{t}") for t in range(CT)]
            wkt_hb = [const.tile([P, D], BF16, tag=f"wkthb{t}", name=f"wkthb{t}") for t in range(CT)]
            wmt_hb = [const.tile([P, C], BF16, tag=f"wmthb{t}", name=f"wmthb{t}") for t in range(CT)]
            for t in range(CT):
                nc.vector.tensor_copy(out=wqt_hb[t], in_=wqt_sb[t])
                nc.vector.tensor_copy(out=wkt_hb[t], in_=wkt_sb[t])
            # residual correction tiles (DMAs emitted after the stats pass)
            xc_sb = [const.tile([P, NQ], BF16, tag=f"xc{t}", name=f"xc{t}") for t in range(CT)]
            ones8 = small.tile([P, 2, P], FP8, tag="ones8")
            nc.vector.memset(ones8, 1.0)
            eps_sb = small.tile([GT, 1], F32, tag="eps")
            nc.vector.memset(eps_sb, EPS)
            mln4_sb = small.tile([P, 1], F32, tag="mln4")
            nc.vector.memset(mln4_sb, -math.log(4.0))
            # load the sqrt_and_others ACT table up front: it also serves the
            # stats' copy/square, so the GN sqrt later needs no table switch
            tbl_scr = small.tile([GT, 1], F32, tag="tblscr")
            nc.scalar.activation(out=tbl_scr, in_=eps_sb,
                                 func=mybir.ActivationFunctionType.Sqrt)

            xqb = [const.tile([P, NQ], F32, tag=f"xqb{t}", name=f"xqb{t}") for t in range(CT)]
            scale_sb = [small.tile([P, 1], F32, tag=f"scale{t}", name=f"scale{t}") for t in range(CT)]
            shift_sb = [small.tile([P, 1], F32, tag=f"shift{t}", name=f"shift{t}") for t in range(CT)]

            # ---- GroupNorm stats overlapped with the x DMA ----
            # Both channel tiles go through ONE group-combine matmul pair to
            # minimize serial DVE<->PE<->Scalar ping-pong after the last stat.
            with tc.tile_pool(name="gn", bufs=2) as gn, \
                 tc.tile_pool(name="gnps", bufs=1, space="PSUM") as gnps, \
                 tc.tile_pool(name="warm", bufs=1, space="PSUM") as wps:
                # PE warmup: the HAM clock gate holds the PE at 1.2 GHz until
                # it sees ~3.4us of sustained activity, and re-throttles after
                # ~3.4us of idle. Burn dummy matmuls during the DMA/GN wait so
                # the attention stream starts (and stays) at 2.4 GHz.
                wtile = wps.tile([P, 2, P], F32, tag="warm")
                wmov = ones8[:, :, :].rearrange("p a b -> p (a b)")
                wout = wtile[:, :, :].rearrange("p a b -> p (a b)")

                def warm_mm(n=1):
                    for _ in range(n):
                        nc.tensor.matmul(wout, ones8[:, 0, :], wmov,
                                         start=True, stop=True)

                warm_mm(16)
                # stats split across engines: the DVE (bn_stats) takes tile 0
                # plus the late halves of tile 1; the slower scalar-accumulate
                # path takes tile 1's first pieces — neither engine alone
                # keeps pace with the x8 DMA.  bn_stats is HW-capped at 512
                # free, so iterate 512-token halves within each dma piece.
                NPC = N // CH
                HPP = TP8 // CH  # 512-halves per dma piece
                SCN = 4  # halves handled by the scalar engine
                mx_both = gn.tile([P, CT, 2], F32, tag="mxb")
                stats = gn.tile([P, NPC, nc.vector.BN_STATS_DIM], F32, tag="st")
                stat1 = gn.tile([P, NPC - SCN, nc.vector.BN_STATS_DIM], F32, tag="st1")
                part = gn.tile([P, 2, SCN], F32, tag="part")
                for cn in range(NPC):
                    pc, hh = divmod(cn, HPP)
                    hsl = slice(hh * CH, (hh + 1) * CH)
                    if cn < SCN:
                        scr = gn.tile([P, CH], F32, tag="scr")
                        nc.scalar.activation(
                            out=scr, in_=x8[:, pc, 1, hsl],
                            func=mybir.ActivationFunctionType.Copy,
                            accum_out=part[:, 0, cn:cn + 1])
                        scr2 = gn.tile([P, CH], F32, tag="scr")
                        nc.scalar.activation(
                            out=scr2, in_=x8[:, pc, 1, hsl],
                            func=mybir.ActivationFunctionType.Square,
                            accum_out=part[:, 1, cn:cn + 1])
                    nc.vector.bn_stats(out=stats[:, cn, :], in_=x8[:, pc, 0, hsl])
                    if cn >= SCN:
                        nc.vector.bn_stats(
                            out=stat1[:, cn - SCN, :], in_=x8[:, pc, 1, hsl])
                    if cn % 2 == 1:
                        warm_mm(2)  # HAM heartbeat while PE waits on GN
                # big non-critical transfers start only now: x8t on the scalar
                # queue (first consumer is ~10us out), wmt/xc on the gpsimd
                # SWDGE queue (slow start, consumers are ~25us out)
                for t in range(CT):
                    nc.scalar.dma_start(
                        out=x8t[:, t * (NMB // 2):(t + 1) * (NMB // 2), :],
                        in_=x8t_ext[:, t * (NMB // 2) * C:(t + 1) * (NMB // 2) * C].rearrange(
                            "p (m c) -> p m c", m=NMB // 2))
                for t in range(CT):
                    nc.gpsimd.dma_start(
                        out=wmt_sb[t], in_=wmt_ext[t * P:(t + 1) * P, :])
                    nc.gpsimd.dma_start(
                        out=xc_sb[t], in_=xc_ext[t * P:(t + 1) * P, :])
                nc.vector.bn_aggr(out=mx_both[:, 0, :], in_=stats)
                # in place: var -> E[x^2] = var + mean^2
                nc.vector.scalar_tensor_tensor(
                    out=mx_both[:, 0, 1:2], in0=mx_both[:, 0, 0:1],
                    scalar=mx_both[:, 0, 0:1], in1=mx_both[:, 0, 1:2],
                    op0=mybir.AluOpType.mult, op1=mybir.AluOpType.add)
                # tile 1: combine the DVE half (mean/var of the back pieces)
                # with the scalar partial sums of the front pieces
                mv1 = gn.tile([P, 2], F32, tag="mv1")
                nc.vector.bn_aggr(out=mv1, in_=stat1)
                nc.vector.scalar_tensor_tensor(
                    out=mv1[:, 1:2], in0=mv1[:, 0:1], scalar=mv1[:, 0:1],
                    in1=mv1[:, 1:2],
                    op0=mybir.AluOpType.mult, op1=mybir.AluOpType.add)
                tots = gn.tile([P, 2], F32, tag="tots")
                nc.vector.reduce_sum(out=tots, in_=part, axis=mybir.AxisListType.X)
                wD = (NPC - SCN) / NPC   # weight of the DVE half
                wS = 1.0 / (SCN * CH) * (SCN / NPC)  # partial-sum scale
                nc.vector.tensor_scalar_mul(
                    out=mx_both[:, 1, :], in0=mv1, scalar1=wD)
                nc.vector.scalar_tensor_tensor(
                    out=mx_both[:, 1, :], in0=tots, scalar=wS,
                    in1=mx_both[:, 1, :],
                    op0=mybir.AluOpType.mult, op1=mybir.AluOpType.add)

                warm_mm(2)
                gps = gnps.tile([GT, CT, 2], F32, tag="gps")
                nc.tensor.matmul(
                    gps[:, :, :].rearrange("p a b -> p (a b)"), ind16_sb,
                    mx_both[:, :, :].rearrange("p a b -> p (a b)"),
                    start=True, stop=True)
                warm_mm(2)
                gsb = gn.tile([GT, CT, 2], F32, tag="gsb")
                nc.vector.tensor_copy(out=gsb, in_=gps)
                vneg = gn.tile([GT, CT, 1], F32, tag="vneg")
                nc.vector.tensor_mul(out=vneg, in0=gsb[:, :, 0:1], in1=gsb[:, :, 0:1])
                nc.vector.tensor_sub(out=vneg, in0=vneg, in1=gsb[:, :, 1:2])
                sd = gn.tile([GT, CT, 1], F32, tag="sd")
                nc.scalar.activation(
                    out=sd, in_=vneg,
                    func=mybir.ActivationFunctionType.Sqrt,
                    bias=eps_sb, scale=-1.0,
                )
                # dummy exp: pull the EXP activation-table load off the
                # first attention group's critical path
                dmy = gn.tile([GT, CT, 1], F32, tag="dmy")
                nc.scalar.activation(
                    out=dmy, in_=vneg,
                    func=mybir.ActivationFunctionType.Exp, scale=1.0)
                g2 = gn.tile([GT, CT, 2], F32, tag="g2")
                nc.vector.tensor_copy(out=g2[:, :, 0:1], in_=gsb[:, :, 0:1])
                nc.vector.reciprocal(out=g2[:, :, 1:2], in_=sd)

                bc = gnps.tile([P, CT, 2], F32, tag="bc")
                nc.tensor.matmul(
                    bc[:, :, :].rearrange("p a b -> p (a b)"), indb_sb,
                    g2[:, :, :].rearrange("p a b -> p (a b)"),
                    start=True, stop=True)
                for t in range(CT):
                    nc.vector.tensor_mul(out=scale_sb[t], in0=gamma_sb[t], in1=bc[:, t, 1:2])
                    sh1 = gn.tile([P, 1], F32, tag="sh1")
                    nc.vector.tensor_mul(out=sh1, in0=bc[:, t, 0:1], in1=scale_sb[t])
                    nc.vector.tensor_sub(out=shift_sb[t], in0=beta_sb[t], in1=sh1)

                # ---- scaled weights + effective biases ----
                # wqk8: [wq'|wk'|wq'|wk'] packed stationary (2x replicated),
                # fp8, both channel tiles stacked for DoubleRow
                wqk8 = const.tile([P, CT, 4 * D], FP8, tag="wqk8", name="wqk8")
                # W'8 = (64 wp wv)^T diag-scaled, fp8, [c-part, ctile, f]
                # (its DVE scaling is emitted after the qk evacuations)
                w8 = const.tile([P, CT, C], FP8, tag="w8", name="w8")
                for t in range(CT):
                    for j in range(2):
                        nc.vector.tensor_scalar_mul(
                            out=wqk8[:, t, (2 * j) * D:(2 * j + 1) * D],
                            in0=wqt_sb[t], scalar1=scale_sb[t])
                        nc.vector.tensor_scalar_mul(
                            out=wqk8[:, t, (2 * j + 1) * D:(2 * j + 2) * D],
                            in0=wkt_sb[t], scalar1=scale_sb[t])
                shift_hb = [small.tile([P, 1], BF16, tag=f"shifthb{t}", name=f"shifthb{t}") for t in range(CT)]
                for t in range(CT):
                    nc.vector.tensor_copy(out=shift_hb[t], in_=shift_sb[t])

                with tc.tile_pool(name="bps", bufs=1, space="PSUM") as bps:
                    bq_eff = small.tile([D, 1], F32, tag="bqe")
                    bk_eff = small.tile([D, 1], F32, tag="bke")
                    psq = bps.tile([D, 1], F32, tag="pq")
                    psk = bps.tile([D, 1], F32, tag="pk")
                    for t in range(CT):
                        nc.tensor.matmul(psq, wqt_hb[t], shift_hb[t], start=(t == 0), stop=(t == CT - 1))
                        nc.tensor.matmul(psk, wkt_hb[t], shift_hb[t], start=(t == 0), stop=(t == CT - 1))
                    nc.vector.tensor_add(out=bq_eff, in0=psq, in1=bq_sb)
                    nc.vector.tensor_add(out=bk_eff, in0=psk, in1=bk_sb)
                    # interleaved bias vector [bq|bk|bq|bk] for the packed evac
                    qkbias = small.tile([P, 1], F32, tag="qkbias")
                    for j in range(2):
                        nc.vector.tensor_copy(out=qkbias[(2 * j) * D:(2 * j + 1) * D, :], in_=bq_eff)
                        nc.vector.tensor_copy(out=qkbias[(2 * j + 1) * D:(2 * j + 2) * D, :], in_=bk_eff)
                # ball = (1/64) (64 W) shift + (wp bv + bp)  (chains after qk)
                ball_sb = [small.tile([P, 1], F32, tag=f"ball{f}", name=f"ball{f}") for f in range(CT)]

            # ---- q/k (packed, 4x-replicated for 4-band S^T, fp8 DR) ----
            # 32-row-band matmuls at different tile_positions run
            # CONCURRENTLY on the PE (16x 32x32 sub-arrays): with q/k
            # replicated on all four 32-row bands, FOUR m-blocks of S^T
            # stream in one matmul-duration instead of two.
            q_rep = const.tile([P, NQ], BF16, tag="qrep")
            k_rep = const.tile([P, N], BF16, tag="krep")
            qkraw = const.tile([P, N], BF16, tag="qkraw")
            with tc.tile_pool(name="qkps", bufs=4, space="PSUM") as qkps:
                # DMA batches: chunk 0 and 1 rearranged immediately (S^T of
                # the first groups waits on them), the rest in wider batches
                batches = [(0, 1), (1, 2), (2, 4), (4, 6), (6, 8)]
                for lo, hi in batches:
                    for cn in range(lo, hi):
                        ns = slice(cn * CH, (cn + 1) * CH)
                        hsl = slice((cn % HPP) * CH, (cn % HPP + 1) * CH)
                        qkp = qkps.tile([P, CH], F32, tag="qkp", name=f"qkp{cn}")
                        nc.tensor.matmul(qkp, wqk8, x8[:, cn // HPP, :, hsl],
                                         start=True, stop=True, perf_mode=DR)
                        nc.vector.tensor_scalar_add(out=qkraw[:, ns], in0=qkp, scalar1=qkbias)
                    # partition rearrange: qkraw q bands {0-31,64-95},
                    # k bands {32-63,96-127} -> 4 copies each
                    bs = slice(lo * CH, hi * CH)
                    for j in range(4):
                        js = slice(32 * j, 32 * (j + 1))
                        (nc.sync if j < 2 else nc.scalar).dma_start(
                            out=k_rep[js, bs],
                            in_=qkraw[32 + 64 * (j % 2):64 + 64 * (j % 2), bs])
                        if hi <= NQ // CH:
                            nc.gpsimd.dma_start(
                                out=q_rep[js, bs],
                                in_=qkraw[64 * (j % 2):32 + 64 * (j % 2), bs])

            # W'8 scaling + ball chain (off the qk critical path):
            # ball[f] = (1/64) * sum_e (64 W)^T[e, f]^T shift[e] + bfix[f]
            with tc.tile_pool(name="bps2", bufs=1, space="PSUM") as bps2:
                for t in range(CT):
                    nc.vector.tensor_scalar_mul(out=w8[:, t, :], in0=wmt_sb[t], scalar1=scale_sb[t])
                    nc.vector.tensor_copy(out=wmt_hb[t], in_=wmt_sb[t])
                for f in range(CT):
                    ps4 = bps2.tile([P, 1], F32, tag=f"pp{f}", name=f"psp{f}")
                    for e in range(CT):
                        nc.tensor.matmul(
                            ps4, wmt_hb[e][:, f * P:(f + 1) * P], shift_hb[e],
                            start=(e == 0), stop=(e == CT - 1),
                        )
                    nc.vector.scalar_tensor_tensor(
                        out=ball_sb[f], in0=ps4, scalar=1.0 / 64.0,
                        in1=bfix_sb[f],
                        op0=mybir.AluOpType.mult, op1=mybir.AluOpType.add)

            # ---- attention ----
            with tc.tile_pool(name="stps", bufs=2, space="PSUM") as stps, \
                 tc.tile_pool(name="attps", bufs=1, space="PSUM") as attps, \
                 tc.tile_pool(name="rsps", bufs=1, space="PSUM") as rsps, \
                 tc.tile_pool(name="flex", bufs=1, space="PSUM") as flex, \
                 tc.tile_pool(name="pp", bufs=6) as pp, \
                 tc.tile_pool(name="attsb", bufs=4) as attsb, \
                 tc.tile_pool(name="osb", bufs=4) as osb, \
                 tc.tile_pool(name="rsb", bufs=2) as rsb:
                pend = None  # deferred epilogue payload of the previous chunk

                def eager_epilogue(ns_p, att2_p, rs_p):
                    """Emitted right at chunk end: frees the xP/rs PSUM banks
                    as fast as possible for the next chunk. The DoubleRow
                    ones-matmul already replicated the denominator on every
                    partition, so one approx-reciprocal pass gives the full
                    broadcast 1/den, and the softmax normalization folds into
                    the xP evacuation itself (fp8 out for the W'8 matmul)."""
                    rec_bc = rsb.tile([P, CH], F32, tag="recbc")
                    nc.vector.reciprocal_approx_fast(out=rec_bc, in_=rs_p)
                    att_sb2 = attsb.tile([P, CT, CH], FP8, tag="attsb2")
                    for e in range(CT):
                        nc.vector.tensor_mul(
                            out=att_sb2[:, e, :], in0=att2_p[:, e, :], in1=rec_bc)
                    return (ns_p, att_sb2)

                def emit_epilogue_f(ep, f, final=False):
                    ns_p, att_sb2 = ep
                    fs = slice(f * P, (f + 1) * P)
                    pjt = flex.tile([P, 2, CH // 2], F32, tag="flex", name=f"pj{f}")
                    pj = pjt[:, :, :].rearrange("p a b -> p (a b)")
                    # ONE DoubleRow matmul contracts both channel tiles
                    nc.tensor.matmul(
                        pj, w8[:, :, fs], att_sb2,
                        start=True, stop=True, perf_mode=DR,
                    )
                    o = osb.tile([P, CH], F32, tag="o")
                    # undo the x64 W' pre-scale during the residual add
                    nc.vector.scalar_tensor_tensor(
                        out=o, in0=pj, scalar=1.0 / 64.0, in1=xqb[f][:, ns_p],
                        op0=mybir.AluOpType.mult, op1=mybir.AluOpType.add)
                    # split each strip across queues so the tail drains fast
                    # (keep gpsimd out of the tail: its queue drain gates
                    # kernel teardown)
                    hc = CH // 2
                    oeng = [nc.sync, nc.scalar] if final else [nc.sync, nc.gpsimd]
                    ne = len(oeng)
                    for hh in range(2):
                        cs2 = slice(ns_p.start + hh * hc, ns_p.start + (hh + 1) * hc)
                        oeng[(2 * f + hh) % ne].dma_start(
                            out=out_ext[fs, cs2], in_=o[:, hh * hc:(hh + 1) * hc])

                # flattened (chunk, group) stream: the S^T/exp front runs LAG
                # groups ahead of the rs/xP drain, ACROSS chunk boundaries,
                # so the PE never sits behind the trailing xP of a chunk
                LAG = 2
                TOT = NCH * NG2
                att2_t = [None] * NCH
                rs_t = [None] * NCH
                p_tiles = [None] * TOT
                st_tiles = [None] * TOT
                for k in range(TOT + LAG):
                    if k < TOT:
                        ch, g = divmod(k, NG2)
                        ns = slice(ch * CH, (ch + 1) * CH)
                        if g == 0:
                            att2_t[ch] = attps.tile([P, CT, CH], F32, tag="att2", name=f"att2c{ch}")
                            rs_t[ch] = rsps.tile([P, CH], F32, tag="rs", name=f"rsc{ch}")
                        if k % 2 == 0:
                            # 4 row-banded S^T matmuls for groups k, k+1 —
                            # all four stream concurrently on the PE
                            st_tiles[k] = stps.tile([P, 2, CH], F32, tag="stg", name=f"stg{k}")
                            st_tiles[k + 1] = stps.tile([P, 2, CH], F32, tag="stg", name=f"stg{k + 1}")
                            for j in range(4):
                                mb = g * 2 + j
                                nc.tensor.matmul(
                                    st_tiles[k + j // 2][:, j % 2, :],
                                    k_rep[32 * j:32 * (j + 1), mb * MB:(mb + 1) * MB],
                                    q_rep[32 * j:32 * (j + 1), ns],
                                    start=True, stop=True,
                                    tile_position=(32 * j, 0),
                                )
                        stg = st_tiles[k]
                        st_tiles[k] = None
                        if ch == 1 and g < CT:
                            # residual base: fp8 x + bf16 correction +
                            # folded bias (DVE has slack here)
                            f = g
                            npq = NQ // TP8  # query-half dma pieces
                            nc.vector.scalar_tensor_tensor(
                                out=xqb[f][:, :].rearrange("p (a b) -> p a b", a=npq),
                                in0=x8[:, 0:npq, f, :], scalar=ball_sb[f],
                                in1=xc_sb[f][:, :].rearrange("p (a b) -> p a b", a=npq),
                                op0=mybir.AluOpType.add, op1=mybir.AluOpType.add)
                        pg = pp.tile([P, 2, CH], FP8, tag="pg")
                        if (k % NG2) in DVE_EXP_STEPS:
                            # Schraudolph: fp8e4m3 bits are linear in log2 —
                            # int8(s*A + B) viewed as fp8 IS exp(s*SM)/4
                            nc.vector.tensor_scalar(
                                out=pg.bitcast(I8), in0=stg,
                                scalar1=SCH_A, scalar2=SCH_B,
                                op0=mybir.AluOpType.mult,
                                op1=mybir.AluOpType.add)
                        else:
                            # exp(s*SM)/4 on ACT (the /4 matches the
                            # Schraudolph tiles; num/den share it)
                            nc.scalar.activation(
                                out=pg, in_=stg,
                                func=mybir.ActivationFunctionType.Exp,
                                scale=SM_SCALE, bias=mln4_sb,
                            )
                        p_tiles[k] = pg
                        # deferred epilogue of the previous chunk, one output
                        # strip per group so it never swamps one group's slack
                        if g in (4, 5) and pend is not None:
                            emit_epilogue_f(pend, g - 4)
                            if g == 5:
                                pend = None
                    if k >= LAG:
                        kp = k - LAG
                        chp, gp = divmod(kp, NG2)
                        pg = p_tiles[kp]
                        p_tiles[kp] = None
                        # denominator: fp8 DoubleRow ones-matmul, result
                        # replicated across all 128 PSUM partitions
                        nc.tensor.matmul(
                            rs_t[chp], ones8, pg,
                            start=(gp == 0), stop=(gp == NG2 - 1),
                            perf_mode=DR,
                        )
                        # x@P: fp8 DoubleRow, two m-blocks per pass,
                        # token-major x8t stationary
                        for e in range(CT):
                            nc.tensor.matmul(
                                att2_t[chp][:, e, :],
                                x8t[:, 2 * gp:2 * gp + 2, e * P:(e + 1) * P],
                                pg,
                                start=(gp == 0), stop=(gp == NG2 - 1),
                                perf_mode=DR,
                            )
                        if gp == NG2 - 1:
                            nsp = slice(chp * CH, (chp + 1) * CH)
                            pend = eager_epilogue(nsp, att2_t[chp], rs_t[chp])
                for f in range(CT):
                    emit_epilogue_f(pend, f, final=True)

    nc.compile()
    _CACHE["nc"] = nc
    return nc




def kernel(x, gamma, beta, wq, bq, wk, bk, wv, bv, wp, bp):
    x = np.ascontiguousarray(np.asarray(x, dtype=np.float32))
    nc = _build()

    GT = G // CT
    ind16 = np.zeros((P, GT), np.float32)
    for c in range(P):
        ind16[c, c // GS] = 1.0 / GS
    indb = np.zeros((GT, P), np.float32)
    for c in range(P):
        indb[c // GS, c] = 1.0

    wm = (np.asarray(wp, np.float64) @ np.asarray(wv, np.float64))
    bfix = (np.asarray(wp, np.float64) @ np.asarray(bv, np.float64)
            + np.asarray(bp, np.float64))
    zc = np.zeros((C,), np.float32)
    common = {
        "wqt": np.ascontiguousarray(np.asarray(wq, np.float32).T),
        "wkt": np.ascontiguousarray(np.asarray(wk, np.float32).T),
        "wmt": np.ascontiguousarray((64.0 * wm).T.astype(np.float32)),
        "gbvp": np.ascontiguousarray(np.stack(
            [np.asarray(gamma, np.float32), np.asarray(beta, np.float32),
             bfix.astype(np.float32), zc], axis=1)),
        "bqk": np.ascontiguousarray(np.stack(
            [np.asarray(bq, np.float32), np.asarray(bk, np.float32)], axis=1)),
        "ind16": ind16,
        "indb": indb,
    }

    xf = x.reshape(B, C, N)
    x8all = xf.astype(ml_dtypes.float8_e4m3)
    # bf16 correction x - fp8(x), exact residual reconstruction on device
    xcall = (xf - x8all.astype(np.float32)).astype(ml_dtypes.bfloat16)
    in_maps = []
    for core in range(8):
        b, half = core // 2, core % 2
        m = dict(common)
        # put this core's query tokens in columns 0:NQ (token order within
        # the key axis is irrelevant to GroupNorm stats and softmax sums)
        hs = slice(half * NQ, (half + 1) * NQ)
        if half == 0:
            xp8 = x8all[b]
        else:
            xp8 = np.concatenate([x8all[b][:, NQ:], x8all[b][:, :NQ]], axis=1)
        # device layout [p, piece, ct, 1024] with channel c = ct*128 + p
        m["x8"] = np.ascontiguousarray(
            xp8.reshape(CT, P, 4, N // 4).transpose(1, 2, 0, 3).reshape(P, N * CT))
        # token-major layout [p=token-in-block, mb, c]
        m["x8t"] = np.ascontiguousarray(
            xp8.reshape(C, NMB, P).transpose(2, 1, 0).reshape(P, NMB * C))
        m["xc"] = np.ascontiguousarray(xcall[b][:, hs])
        in_maps.append(m)

    global _last_in_maps
    _last_in_maps = in_maps
    res = run_bass_kernel_spmd(nc, in_maps, list(range(8)))

    y = np.empty((B, C, N), np.float32)
    for core in range(8):
        b, half = core // 2, core % 2
        y[b][:, half * NQ:(half + 1) * NQ] = res.results[core]["out"]
    return y.reshape(B, C, H, W)


# revision 26
# speedup vs baseline: 1.0947x; 1.0947x over previous
"""AttentionBlock (GroupNorm + 1x1-conv QKV self-attention + residual) on 8 TRN2 cores.

Sharding: data-parallel over batch B=4 x sequence-parallel over the 4096
tokens (2 cores per batch element, each handling 2048 query rows; each core
receives x with ITS query tokens permuted to the front — attention sums and
GroupNorm stats are permutation-invariant over tokens, so one SPMD program
serves both halves).

Key algebraic fold: since softmax columns sum to 1,
  proj = wp @ (wv h + bv 1^T) @ P_norm + bp 1^T
       = (wp wv) @ (h @ P_norm) + (wp bv + bp) 1^T
  h @ P_norm = scale ⊙ (x @ P_norm) + shift 1^T
so with W = wp wv (precomputed on host), W' = W diag(scale):
  proj = W' @ (x @ P)/den + [W shift + wp bv + bp]
The V path (v production, V^T materialization + PSUM->SBUF copy) disappears;
the P*V matmul becomes x^T-stationary (x ships in BOTH channel-major fp8
(x8) and token-major fp8 (x8t) layouts, 1MB each).  W' ships x64 pre-scaled
so its fp8 encoding avoids denormals; the 1/64 folds into the residual add.

exp splits across BOTH engines per tile: the ACT engine computes
exp(s*scale)/4 -> fp8 for m-block 0 (bias=-ln4) while the DVE computes
m-block 1 via the Schraudolph bit trick: fp8e4m3 bits are linear in
log2(v), so int8(s*A + B) bit-viewed as fp8 IS exp(s*scale)/4 to ~±3% —
one tensor_scalar instruction straight from the S^T PSUM.  The two halves
run CONCURRENTLY (~0.7us) instead of one serial 1.1us activation: the
S^T-PSUM ring (2 tiles deep; PSUM is full) is latency-paced by
S^T -> exp -> S^T+2, so halving exp latency shortens every pipeline step.
The /4 keeps the bit pattern clear of the fp8 NaN region with ~6x range
margin (s*scale measured in ±0.8); num and den share the /4 so softmax is
unaffected.

Schedule highlights:
  - ONE packed DMA carries all small constants (indicator matrices, wq/wk,
    biases): separate tiny DMAs each cost a dispatch slot + ~2us HBM
    completion and starved the x8 transfer of queue slots.
  - x8 ships in 4 pieces with 2KB contiguous lines, split across the two
    fast HWDGE queues (sync+scalar), ahead of all bulk; GroupNorm stats
    chase the pieces, split DVE/ACT.
  - PE warmup: the HAM clock gate holds the PE at 1.2 GHz until ~3.4us of
    sustained activity and re-throttles after ~3.4us idle; dummy matmuls
    (data-gated on the x8 pieces so they SPREAD across the lead-in) keep
    the clock at 2.4 GHz from the first real matmul on.
  - q/k: packed [wq'|wk'|wq'|wk'] fp8 DoubleRow matmul per 512-token chunk,
    partition-rearranged into 2x-replicated q_rep/k_rep (bands 0-31/32-63)
    via SBUF->SBUF DMAs; S^T is 2 concurrent row-banded bf16 matmuls (K=32).
  - denominator: fp8 DoubleRow ones-matmul replicates column sums on all
    128 partitions; one reciprocal_approx_fast gives broadcast 1/den.
  - x@P: fp8 DoubleRow, two 128-key m-blocks per pass, x8t stationary.
  - epilogue per chunk: evac (x@P)*(1/den) -> fp8, ONE DoubleRow matmul per
    128-channel strip with W'8, o = pj/64 + (x + xc + ball), DMA out.
  - the (chunk, group) space is one flattened software-pipelined stream
    with the rs/xP drain LAG=2 groups behind the S^T/exp front.
"""
import math
import sys

sys.path.insert(0, "/opt/trn_rl_repo")

import ml_dtypes
import numpy as np

import concourse.tile as tile
from concourse import bacc, mybir
from concourse.bass_utils import run_bass_kernel_spmd

F32 = mybir.dt.float32
BF16 = mybir.dt.bfloat16
FP8 = mybir.dt.float8e4
I8 = mybir.dt.int8

B, C, H, W = 4, 256, 64, 64
N = H * W          # 4096 tokens
NQ = N // 2        # 2048 query rows per core
D = C // 8         # 32 qk dim
G = 32             # groups
GS = C // G        # 8 channels per group
EPS = 1e-5
P = 128            # partitions
CT = C // P        # 2 channel tiles
CH = 512           # nq chunk
NCH = NQ // CH     # 4 chunks
MB = 128           # m block
NMB = N // MB      # 32 m blocks
NG2 = NMB // 2     # 16 groups of 2 m-blocks
SM_SCALE = float(D) ** -0.5
DR = mybir.MatmulPerfMode.DoubleRow

# Schraudolph-to-fp8e4m3: bits = 8*(log2(v) + 7 - CORR), v = exp(s*SM)/4
SCH_CORR = 0.0430
SCH_A = 8.0 * SM_SCALE * math.log2(math.e)
SCH_B = 8.0 * (5.0 - SCH_CORR)

# packed consts layout (columns in the [128, NCONST] f32 block)
CO_IND16 = 0            # [128, 16]
CO_INDB = 16            # [16, 128] in rows 0:16
CO_GBVP0 = 144          # [128, 4]
CO_GBVP1 = 148          # [128, 4]
CO_BQK = 152            # [32, 2] in rows 0:32
CO_WQT0 = 154           # [128, 32]
CO_WQT1 = 186
CO_WKT0 = 218
CO_WKT1 = 250
NCONST = 282

_CACHE = {}
_last_in_maps = None


def _build():
    if "nc" in _CACHE:
        return _CACHE["nc"]

    nc = bacc.Bacc("TRN2", target_bir_lowering=False, debug=False, num_devices=8)

    x8_ext = nc.declare_dram_parameter("x8", [P, N * CT], FP8, isOutput=False)
    x8t_ext = nc.declare_dram_parameter("x8t", [P, NMB * C], FP8, isOutput=False)
    xc_ext = nc.declare_dram_parameter("xc", [C, NQ], BF16, isOutput=False)
    # (64 * wp @ wv)^T
    wmt_ext = nc.declare_dram_parameter("wmt", [C, C], F32, isOutput=False)
    consts_ext = nc.declare_dram_parameter("consts", [P, NCONST], F32, isOutput=False)
    out_ext = nc.declare_dram_parameter("out", [C, NQ], F32, isOutput=True)

    GT = G // CT  # 16 groups per channel tile

    with tile.TileContext(nc) as tc:
        with tc.tile_pool(name="const", bufs=1) as const, \
             tc.tile_pool(name="small", bufs=1) as small:
            # x8 first on both fast HWDGE queues: 4 pieces with 2KB
            # contiguous lines per partition; stats chase the pieces
            NP8 = 4                      # x8 dma pieces
            TP8 = N // NP8               # tokens per piece (1024)
            HPP = TP8 // CH              # 512-halves per piece
            x8 = const.tile([P, NP8, CT, TP8], FP8, tag="x8", name="x8")
            PCH = CT * TP8               # flattened columns per piece
            for pc in range(NP8):
                qeng = nc.sync if pc % 2 == 0 else nc.scalar
                qeng.dma_start(
                    out=x8[:, pc, :, :],
                    in_=x8_ext[:, pc * PCH:(pc + 1) * PCH].rearrange(
                        "p (t b) -> p t b", t=CT))
            # all small constants in ONE transfer right behind x8
            consts_sb = const.tile([P, NCONST], F32, tag="consts", name="consts")
            nc.sync.dma_start(out=consts_sb, in_=consts_ext[:])
            # x8t right behind x8 on the scalar queue (needed ~10us after x8)
            x8t = const.tile([P, NMB, C], FP8, tag="x8t", name="x8t")
            for t in range(CT):
                nc.scalar.dma_start(
                    out=x8t[:, t * (NMB // 2):(t + 1) * (NMB // 2), :],
                    in_=x8t_ext[:, t * (NMB // 2) * C:(t + 1) * (NMB // 2) * C].rearrange(
                        "p (m c) -> p m c", m=NMB // 2))

            ind16_sb = consts_sb[:, CO_IND16:CO_IND16 + GT]
            indb_sb = consts_sb[0:GT, CO_INDB:CO_INDB + P]
            gbvp_sb = [consts_sb[:, CO_GBVP0:CO_GBVP0 + 4],
                       consts_sb[:, CO_GBVP1:CO_GBVP1 + 4]]
            bqk_sb = consts_sb[0:D, CO_BQK:CO_BQK + 2]
            wqt_sb = [consts_sb[:, CO_WQT0:CO_WQT0 + D],
                      consts_sb[:, CO_WQT1:CO_WQT1 + D]]
            wkt_sb = [consts_sb[:, CO_WKT0:CO_WKT0 + D],
                      consts_sb[:, CO_WKT1:CO_WKT1 + D]]
            gamma_sb = [gbvp_sb[t][:, 0:1] for t in range(CT)]
            beta_sb = [gbvp_sb[t][:, 1:2] for t in range(CT)]
            bfix_sb = [gbvp_sb[t][:, 2:3] for t in range(CT)]
            bq_sb = bqk_sb[:, 0:1]
            bk_sb = bqk_sb[:, 1:2]

            # ---- persistent tiles for deferred bulk ----
            wmt_sb = [const.tile([P, C], F32, tag=f"wmt{t}", name=f"wmt{t}")
                      for t in range(CT)]
            # bf16 copies for the (small) effective-bias matmuls
            wqt_hb = [const.tile([P, D], BF16, tag=f"wqthb{t}", name=f"wqthb{t}") for t in range(CT)]
            wkt_hb = [const.tile([P, D], BF16, tag=f"wkthb{t}", name=f"wkthb{t}") for t in range(CT)]
            wmt_hb = [const.tile([P, C], BF16, tag=f"wmthb{t}", name=f"wmthb{t}") for t in range(CT)]
            # residual correction tiles (DMAs emitted after the stats pass)
            xc_sb = [const.tile([P, NQ], BF16, tag=f"xc{t}", name=f"xc{t}") for t in range(CT)]
            ones8 = small.tile([P, 2, P], FP8, tag="ones8")
            nc.vector.memset(ones8, 1.0)
            eps_sb = small.tile([GT, 1], F32, tag="eps")
            nc.vector.memset(eps_sb, EPS)
            mln4_sb = small.tile([P, 1], F32, tag="mln4")
            nc.vector.memset(mln4_sb, -math.log(4.0))
            # load the sqrt_and_others ACT table up front: it also serves the
            # stats' copy/square, so the GN sqrt later needs no table switch
            tbl_scr = small.tile([GT, 1], F32, tag="tblscr")
            nc.scalar.activation(out=tbl_scr, in_=eps_sb,
                                 func=mybir.ActivationFunctionType.Sqrt)

            xqb = [const.tile([P, NQ], F32, tag=f"xqb{t}", name=f"xqb{t}") for t in range(CT)]
            scale_sb = [small.tile([P, 1], F32, tag=f"scale{t}", name=f"scale{t}") for t in range(CT)]
            shift_sb = [small.tile([P, 1], F32, tag=f"shift{t}", name=f"shift{t}") for t in range(CT)]

            # ---- GroupNorm stats overlapped with the x DMA ----
            with tc.tile_pool(name="gn", bufs=2) as gn, \
                 tc.tile_pool(name="gnps", bufs=1, space="PSUM") as gnps, \
                 tc.tile_pool(name="warm", bufs=1, space="PSUM") as wps:
                # PE warmup burst + data-gated heartbeats (see header)
                wtile = wps.tile([P, 2, P], F32, tag="warm")
                wout = wtile[:, :, :].rearrange("p a b -> p (a b)")
                wmov = ones8[:, :, :].rearrange("p a b -> p (a b)")

                def warm_mm(n=1, mov=None):
                    m = mov if mov is not None else wmov
                    fs = m.free_size()
                    for _ in range(n):
                        nc.tensor.matmul(wout[:, 0:fs], ones8[:, 0, :], m,
                                         start=True, stop=True)

                warm_mm(14)
                # stats split across engines: the DVE (bn_stats) takes tile 0
                # plus the late halves of tile 1; the scalar-accumulate path
                # takes tile 1's first halves.  bn_stats is HW-capped at 512
                # free, so iterate 512-token halves within each dma piece.
                NPC = N // CH
                SCN = 4  # halves handled by the scalar engine
                mx_both = gn.tile([P, CT, 2], F32, tag="mxb")
                stats = gn.tile([P, NPC, nc.vector.BN_STATS_DIM], F32, tag="st")
                stat1 = gn.tile([P, NPC - SCN, nc.vector.BN_STATS_DIM], F32, tag="st1")
                part = gn.tile([P, 2, SCN], F32, tag="part")
                for cn in range(NPC):
                    pc, hh = divmod(cn, HPP)
                    hsl = slice(hh * CH, (hh + 1) * CH)
                    if cn < SCN:
                        scr = gn.tile([P, CH], F32, tag="scr")
                        nc.scalar.activation(
                            out=scr, in_=x8[:, pc, 1, hsl],
                            func=mybir.ActivationFunctionType.Copy,
                            accum_out=part[:, 0, cn:cn + 1])
                        scr2 = gn.tile([P, CH], F32, tag="scr")
                        nc.scalar.activation(
                            out=scr2, in_=x8[:, pc, 1, hsl],
                            func=mybir.ActivationFunctionType.Square,
                            accum_out=part[:, 1, cn:cn + 1])
                    nc.vector.bn_stats(out=stats[:, cn, :], in_=x8[:, pc, 0, hsl])
                    if cn >= SCN:
                        nc.vector.bn_stats(
                            out=stat1[:, cn - SCN, :], in_=x8[:, pc, 1, hsl])
                    if hh == HPP - 1:
                        # heartbeat gated on this x8 piece: spreads the dummy
                        # matmuls across the DMA wait instead of bunching
                        warm_mm(3, mov=x8[:, pc, 0, 0:2 * P])
                # big non-critical transfers start only now
                for t in range(CT):
                    nc.sync.dma_start(
                        out=wmt_sb[t], in_=wmt_ext[t * P:(t + 1) * P, :])
                    nc.sync.dma_start(
                        out=xc_sb[t], in_=xc_ext[t * P:(t + 1) * P, :])
                nc.vector.bn_aggr(out=mx_both[:, 0, :], in_=stats)
                # in place: var -> E[x^2] = var + mean^2
                nc.vector.scalar_tensor_tensor(
                    out=mx_both[:, 0, 1:2], in0=mx_both[:, 0, 0:1],
                    scalar=mx_both[:, 0, 0:1], in1=mx_both[:, 0, 1:2],
                    op0=mybir.AluOpType.mult, op1=mybir.AluOpType.add)
                # tile 1: combine the DVE half with the scalar partial sums
                mv1 = gn.tile([P, 2], F32, tag="mv1")
                nc.vector.bn_aggr(out=mv1, in_=stat1)
                nc.vector.scalar_tensor_tensor(
                    out=mv1[:, 1:2], in0=mv1[:, 0:1], scalar=mv1[:, 0:1],
                    in1=mv1[:, 1:2],
                    op0=mybir.AluOpType.mult, op1=mybir.AluOpType.add)
                tots = gn.tile([P, 2], F32, tag="tots")
                nc.vector.reduce_sum(out=tots, in_=part, axis=mybir.AxisListType.X)
                wD = (NPC - SCN) / NPC   # weight of the DVE half
                wS = 1.0 / (SCN * CH) * (SCN / NPC)  # partial-sum scale
                nc.vector.tensor_scalar_mul(
                    out=mx_both[:, 1, :], in0=mv1, scalar1=wD)
                nc.vector.scalar_tensor_tensor(
                    out=mx_both[:, 1, :], in0=tots, scalar=wS,
                    in1=mx_both[:, 1, :],
                    op0=mybir.AluOpType.mult, op1=mybir.AluOpType.add)

                warm_mm(2, mov=x8[:, NP8 - 1, 1, 0:2 * P])
                gps = gnps.tile([GT, CT, 2], F32, tag="gps")
                nc.tensor.matmul(
                    gps[:, :, :].rearrange("p a b -> p (a b)"), ind16_sb,
                    mx_both[:, :, :].rearrange("p a b -> p (a b)"),
                    start=True, stop=True)
                warm_mm(2, mov=x8[:, NP8 - 1, 1, 2 * P:4 * P])
                gsb = gn.tile([GT, CT, 2], F32, tag="gsb")
                nc.vector.tensor_copy(out=gsb, in_=gps)
                vneg = gn.tile([GT, CT, 1], F32, tag="vneg")
                nc.vector.tensor_mul(out=vneg, in0=gsb[:, :, 0:1], in1=gsb[:, :, 0:1])
                nc.vector.tensor_sub(out=vneg, in0=vneg, in1=gsb[:, :, 1:2])
                sd = gn.tile([GT, CT, 1], F32, tag="sd")
                nc.scalar.activation(
                    out=sd, in_=vneg,
                    func=mybir.ActivationFunctionType.Sqrt,
                    bias=eps_sb, scale=-1.0,
                )
                # switch the ACT table to the exp set now, while the DVE
                # finishes the GN chain — off the first group's critical path
                dmy = gn.tile([GT, CT, 1], F32, tag="dmy")
                nc.scalar.activation(
                    out=dmy, in_=vneg,
                    func=mybir.ActivationFunctionType.Exp, scale=1.0)
                g2 = gn.tile([GT, CT, 2], F32, tag="g2")
                nc.vector.tensor_copy(out=g2[:, :, 0:1], in_=gsb[:, :, 0:1])
                nc.vector.reciprocal(out=g2[:, :, 1:2], in_=sd)

                bc = gnps.tile([P, CT, 2], F32, tag="bc")
                nc.tensor.matmul(
                    bc[:, :, :].rearrange("p a b -> p (a b)"), indb_sb,
                    g2[:, :, :].rearrange("p a b -> p (a b)"),
                    start=True, stop=True)
                for t in range(CT):
                    nc.vector.tensor_mul(out=scale_sb[t], in0=gamma_sb[t], in1=bc[:, t, 1:2])
                    sh1 = gn.tile([P, 1], F32, tag="sh1")
                    nc.vector.tensor_mul(out=sh1, in0=bc[:, t, 0:1], in1=scale_sb[t])
                    nc.vector.tensor_sub(out=shift_sb[t], in0=beta_sb[t], in1=sh1)

                # ---- scaled weights + effective biases ----
                # wqk8: [wq'|wk'|wq'|wk'] packed stationary (2x replicated),
                # fp8, both channel tiles stacked for DoubleRow
                wqk8 = const.tile([P, CT, 4 * D], FP8, tag="wqk8", name="wqk8")
                # W'8 = (64 wp wv)^T diag-scaled, fp8, [c-part, ctile, f]
                # (its DVE scaling is emitted after the qk evacuations)
                w8 = const.tile([P, CT, C], FP8, tag="w8", name="w8")
                for t in range(CT):
                    for j in range(2):
                        nc.vector.tensor_scalar_mul(
                            out=wqk8[:, t, (2 * j) * D:(2 * j + 1) * D],
                            in0=wqt_sb[t], scalar1=scale_sb[t])
                        nc.vector.tensor_scalar_mul(
                            out=wqk8[:, t, (2 * j + 1) * D:(2 * j + 2) * D],
                            in0=wkt_sb[t], scalar1=scale_sb[t])
                shift_hb = [small.tile([P, 1], BF16, tag=f"shifthb{t}", name=f"shifthb{t}") for t in range(CT)]
                for t in range(CT):
                    nc.vector.tensor_copy(out=wqt_hb[t], in_=wqt_sb[t])
                    nc.vector.tensor_copy(out=wkt_hb[t], in_=wkt_sb[t])
                    nc.vector.tensor_copy(out=shift_hb[t], in_=shift_sb[t])

                with tc.tile_pool(name="bps", bufs=1, space="PSUM") as bps:
                    bq_eff = small.tile([D, 1], F32, tag="bqe")
                    bk_eff = small.tile([D, 1], F32, tag="bke")
                    psq = bps.tile([D, 1], F32, tag="pq")
                    psk = bps.tile([D, 1], F32, tag="pk")
                    for t in range(CT):
                        nc.tensor.matmul(psq, wqt_hb[t], shift_hb[t], start=(t == 0), stop=(t == CT - 1))
                        nc.tensor.matmul(psk, wkt_hb[t], shift_hb[t], start=(t == 0), stop=(t == CT - 1))
                    nc.vector.tensor_add(out=bq_eff, in0=psq, in1=bq_sb)
                    nc.vector.tensor_add(out=bk_eff, in0=psk, in1=bk_sb)
                    # interleaved bias vector [bq|bk|bq|bk] for the packed evac
                    qkbias = small.tile([P, 1], F32, tag="qkbias")
                    for j in range(2):
                        nc.vector.tensor_copy(out=qkbias[(2 * j) * D:(2 * j + 1) * D, :], in_=bq_eff)
                        nc.vector.tensor_copy(out=qkbias[(2 * j + 1) * D:(2 * j + 2) * D, :], in_=bk_eff)
                # ball = (1/64) (64 W) shift + (wp bv + bp)  (chains after qk)
                ball_sb = [small.tile([P, 1], F32, tag=f"ball{f}", name=f"ball{f}") for f in range(CT)]

            # ---- q/k (packed, 2x-replicated, fp8 DoubleRow) ----
            q_rep = const.tile([64, NQ], BF16, tag="qrep")
            k_rep = const.tile([64, N], BF16, tag="krep")
            qkraw = const.tile([P, N], BF16, tag="qkraw")
            with tc.tile_pool(name="qkps", bufs=4, space="PSUM") as qkps:
                # DMA batches: chunk 0 and 1 rearranged immediately (S^T of
                # the first groups waits on them), the rest in wider batches
                batches = [(0, 1), (1, 2), (2, 4), (4, 6), (6, 8)]
                for lo, hi in batches:
                    for cn in range(lo, hi):
                        ns = slice(cn * CH, (cn + 1) * CH)
                        hsl = slice((cn % HPP) * CH, (cn % HPP + 1) * CH)
                        qkp = qkps.tile([P, CH], F32, tag="qkp", name=f"qkp{cn}")
                        nc.tensor.matmul(qkp, wqk8, x8[:, cn // HPP, :, hsl],
                                         start=True, stop=True, perf_mode=DR)
                        nc.vector.tensor_scalar_add(out=qkraw[:, ns], in0=qkp, scalar1=qkbias)
                    # partition rearrange: q bands {0-31,64-95}, k {32-63,96-127}
                    bs = slice(lo * CH, hi * CH)
                    nc.sync.dma_start(out=k_rep[0:32, bs], in_=qkraw[32:64, bs])
                    nc.sync.dma_start(out=k_rep[32:64, bs], in_=qkraw[96:128, bs])
                    if hi <= NQ // CH:
                        nc.gpsimd.dma_start(out=q_rep[0:32, bs], in_=qkraw[0:32, bs])
                        nc.gpsimd.dma_start(out=q_rep[32:64, bs], in_=qkraw[64:96, bs])

            # W'8 scaling + ball chain (off the qk critical path):
            # ball[f] = (1/64) * sum_e (64 W)^T[e, f]^T shift[e] + bfix[f]
            with tc.tile_pool(name="bps2", bufs=1, space="PSUM") as bps2:
                for t in range(CT):
                    nc.vector.tensor_scalar_mul(out=w8[:, t, :], in0=wmt_sb[t], scalar1=scale_sb[t])
                    nc.vector.tensor_copy(out=wmt_hb[t], in_=wmt_sb[t])
                for f in range(CT):
                    ps4 = bps2.tile([P, 1], F32, tag=f"pp{f}", name=f"psp{f}")
                    for e in range(CT):
                        nc.tensor.matmul(
                            ps4, wmt_hb[e][:, f * P:(f + 1) * P], shift_hb[e],
                            start=(e == 0), stop=(e == CT - 1),
                        )
                    nc.vector.scalar_tensor_tensor(
                        out=ball_sb[f], in0=ps4, scalar=1.0 / 64.0,
                        in1=bfix_sb[f],
                        op0=mybir.AluOpType.mult, op1=mybir.AluOpType.add)

            # ---- attention ----
            with tc.tile_pool(name="stps", bufs=2, space="PSUM") as stps, \
                 tc.tile_pool(name="attps", bufs=1, space="PSUM") as attps, \
                 tc.tile_pool(name="rsps", bufs=1, space="PSUM") as rsps, \
                 tc.tile_pool(name="flex", bufs=1, space="PSUM") as flex, \
                 tc.tile_pool(name="pp", bufs=6) as pp, \
                 tc.tile_pool(name="attsb", bufs=4) as attsb, \
                 tc.tile_pool(name="osb", bufs=4) as osb, \
                 tc.tile_pool(name="rsb", bufs=2) as rsb:
                pend = None  # deferred epilogue payload of the previous chunk

                def eager_epilogue(ns_p, att2_p, rs_p):
                    """Emitted right at chunk end: frees the xP/rs PSUM banks
                    as fast as possible for the next chunk."""
                    rec_bc = rsb.tile([P, CH], F32, tag="recbc")
                    nc.vector.reciprocal_approx_fast(out=rec_bc, in_=rs_p)
                    att_sb2 = attsb.tile([P, CT, CH], FP8, tag="attsb2")
                    for e in range(CT):
                        nc.vector.tensor_mul(
                            out=att_sb2[:, e, :], in0=att2_p[:, e, :], in1=rec_bc)
                    return (ns_p, att_sb2)

                def emit_epilogue_f(ep, f, final=False):
                    ns_p, att_sb2 = ep
                    fs = slice(f * P, (f + 1) * P)
                    pjt = flex.tile([P, 2, CH // 2], F32, tag="flex", name=f"pj{f}")
                    pj = pjt[:, :, :].rearrange("p a b -> p (a b)")
                    # ONE DoubleRow matmul contracts both channel tiles
                    nc.tensor.matmul(
                        pj, w8[:, :, fs], att_sb2,
                        start=True, stop=True, perf_mode=DR,
                    )
                    o = osb.tile([P, CH], F32, tag="o")
                    # undo the x64 W' pre-scale during the residual add
                    nc.vector.scalar_tensor_tensor(
                        out=o, in0=pj, scalar=1.0 / 64.0, in1=xqb[f][:, ns_p],
                        op0=mybir.AluOpType.mult, op1=mybir.AluOpType.add)
                    # split each strip across queues so the tail drains fast
                    hc = CH // 2
                    oeng = [nc.sync, nc.scalar] if final else [nc.sync, nc.gpsimd]
                    ne = len(oeng)
                    for hh in range(2):
                        cs2 = slice(ns_p.start + hh * hc, ns_p.start + (hh + 1) * hc)
                        oeng[(2 * f + hh) % ne].dma_start(
                            out=out_ext[fs, cs2], in_=o[:, hh * hc:(hh + 1) * hc])

                LAG = 2
                TOT = NCH * NG2
                att2_t = [None] * NCH
                rs_t = [None] * NCH
                p_tiles = [None] * TOT
                for k in range(TOT + LAG):
                    if k < TOT:
                        ch, g = divmod(k, NG2)
                        ns = slice(ch * CH, (ch + 1) * CH)
                        if g == 0:
                            att2_t[ch] = attps.tile([P, CT, CH], F32, tag="att2", name=f"att2c{ch}")
                            rs_t[ch] = rsps.tile([P, CH], F32, tag="rs", name=f"rsc{ch}")
                        # 2 row-banded S^T matmuls (concurrent on the PE)
                        stg = stps.tile([P, 2, CH], F32, tag="stg")
                        for j in range(2):
                            mb = g * 2 + j
                            nc.tensor.matmul(
                                stg[:, j, :],
                                k_rep[32 * j:32 * (j + 1), mb * MB:(mb + 1) * MB],
                                q_rep[32 * j:32 * (j + 1), ns],
                                start=True, stop=True,
                                tile_position=(32 * j, 0),
                            )
                        if ch == 1 and g < CT:
                            # residual base: fp8 x + bf16 correction + folded
                            # bias (DVE has slack here)
                            f = g
                            npq = NQ // TP8
                            nc.vector.scalar_tensor_tensor(
                                out=xqb[f][:, :].rearrange("p (a b) -> p a b", a=npq),
                                in0=x8[:, 0:npq, f, :], scalar=ball_sb[f],
                                in1=xc_sb[f][:, :].rearrange("p (a b) -> p a b", a=npq),
                                op0=mybir.AluOpType.add, op1=mybir.AluOpType.add)
                        # exp split across BOTH engines, halves run
                        # concurrently: ACT takes m-block 0, DVE (Schraudolph
                        # bit trick) m-block 1
                        pg = pp.tile([P, 2, CH], FP8, tag="pg")
                        nc.scalar.activation(
                            out=pg[:, 0, :], in_=stg[:, 0, :],
                            func=mybir.ActivationFunctionType.Exp,
                            scale=SM_SCALE, bias=mln4_sb,
                        )
                        nc.vector.tensor_scalar(
                            out=pg[:, 1, :].bitcast(I8), in0=stg[:, 1, :],
                            scalar1=SCH_A, scalar2=SCH_B,
                            op0=mybir.AluOpType.mult,
                            op1=mybir.AluOpType.add)
                        p_tiles[k] = pg
                        # deferred epilogue of the previous chunk, one output
                        # strip per group so it never swamps one group's slack
                        if g in (4, 5) and pend is not None:
                            emit_epilogue_f(pend, g - 4)
                            if g == 5:
                                pend = None
                    if k >= LAG:
                        kp = k - LAG
                        chp, gp = divmod(kp, NG2)
                        pg = p_tiles[kp]
                        p_tiles[kp] = None
                        # denominator: fp8 DoubleRow ones-matmul, result
                        # replicated across all 128 PSUM partitions
                        nc.tensor.matmul(
                            rs_t[chp], ones8, pg,
                            start=(gp == 0), stop=(gp == NG2 - 1),
                            perf_mode=DR,
                        )
                        # x@P: fp8 DoubleRow, two m-blocks per pass,
                        # token-major x8t stationary
                        for e in range(CT):
                            nc.tensor.matmul(
                                att2_t[chp][:, e, :],
                                x8t[:, 2 * gp:2 * gp + 2, e * P:(e + 1) * P],
                                pg,
                                start=(gp == 0), stop=(gp == NG2 - 1),
                                perf_mode=DR,
                            )
                        if gp == NG2 - 1:
                            nsp = slice(chp * CH, (chp + 1) * CH)
                            pend = eager_epilogue(nsp, att2_t[chp], rs_t[chp])
                for f in range(CT):
                    emit_epilogue_f(pend, f, final=True)

    nc.compile()
    _CACHE["nc"] = nc
    return nc




def kernel(x, gamma, beta, wq, bq, wk, bk, wv, bv, wp, bp):
    x = np.ascontiguousarray(np.asarray(x, dtype=np.float32))
    nc = _build()

    GT = G // CT
    consts = np.zeros((P, NCONST), np.float32)
    # ind16[c, c//8] = 1/8 (group-average indicator, per 128-channel tile);
    # indb[g, c] = (c//8 == g) broadcasts group rows back to channels — the
    # same matrix serves both channel tiles (tile index rides in the [GT,
    # CT, 2] layout of the combine matmuls).
    for c in range(P):
        consts[c, CO_IND16 + c // GS] = 1.0 / GS
        consts[c // GS, CO_INDB + c] = 1.0
    consts[0:D, CO_BQK + 0] = np.asarray(bq, np.float32)
    consts[0:D, CO_BQK + 1] = np.asarray(bk, np.float32)
    wqT = np.asarray(wq, np.float32).T
    wkT = np.asarray(wk, np.float32).T
    consts[:, CO_WQT0:CO_WQT0 + D] = wqT[0:P]
    consts[:, CO_WQT1:CO_WQT1 + D] = wqT[P:C]
    consts[:, CO_WKT0:CO_WKT0 + D] = wkT[0:P]
    consts[:, CO_WKT1:CO_WKT1 + D] = wkT[P:C]

    wm = (np.asarray(wp, np.float64) @ np.asarray(wv, np.float64))
    bfix = (np.asarray(wp, np.float64) @ np.asarray(bv, np.float64)
            + np.asarray(bp, np.float64))
    consts[:, CO_GBVP0:CO_GBVP0 + 4] = np.stack(
        [np.asarray(gamma, np.float32)[0:P], np.asarray(beta, np.float32)[0:P],
         bfix.astype(np.float32)[0:P], np.zeros(P, np.float32)], axis=1)
    consts[:, CO_GBVP1:CO_GBVP1 + 4] = np.stack(
        [np.asarray(gamma, np.float32)[P:C], np.asarray(beta, np.float32)[P:C],
         bfix.astype(np.float32)[P:C], np.zeros(P, np.float32)], axis=1)

    common = {
        "wmt": np.ascontiguousarray((64.0 * wm).T.astype(np.float32)),
        "consts": np.ascontiguousarray(consts),
    }

    xf = x.reshape(B, C, N)
    x8all = xf.astype(ml_dtypes.float8_e4m3)
    # bf16 correction x - fp8(x), exact residual reconstruction on device
    xcall = (xf - x8all.astype(np.float32)).astype(ml_dtypes.bfloat16)
    in_maps = []
    for core in range(8):
        b, half = core // 2, core % 2
        m = dict(common)
        hs = slice(half * NQ, (half + 1) * NQ)
        if half == 0:
            xp8 = x8all[b]
        else:
            xp8 = np.concatenate([x8all[b][:, NQ:], x8all[b][:, :NQ]], axis=1)
        # device layout [p, piece, ct, 1024] with channel c = ct*128 + p
        m["x8"] = np.ascontiguousarray(
            xp8.reshape(CT, P, 4, N // 4).transpose(1, 2, 0, 3).reshape(P, N * CT))
        # token-major layout [p=token-in-block, mb, c]
        m["x8t"] = np.ascontiguousarray(
            xp8.reshape(C, NMB, P).transpose(2, 1, 0).reshape(P, NMB * C))
        m["xc"] = np.ascontiguousarray(xcall[b][:, hs])
        in_maps.append(m)

    global _last_in_maps
    _last_in_maps = in_maps
    res = run_bass_kernel_spmd(nc, in_maps, list(range(8)))

    y = np.empty((B, C, N), np.float32)
    for core in range(8):
        b, half = core // 2, core % 2
        y[b][:, half * NQ:(half + 1) * NQ] = res.results[core]["out"]
    return y.reshape(B, C, H, W)
